# revision 3
# baseline (speedup 1.0000x reference)
"""Causal self-attention (B=4, L=2048, D=1024, H=16) on 8 Trainium2 NeuronCores.

Sharding: core c -> (batch b = c//2, head-group g = c%2 of 8 heads).
Each core computes the qkv projection for its 8 heads, causal attention, and a
partial out-projection (its head-group's rows of W_out). The host sums the two
partials per batch and adds biases (exact: out-proj is linear and the v-bias
passes through softmax-weighted averaging).

v2 — single fused software pipeline (vs v1's 3 sequential phases):
 - Stage s (q-block of 512 rows): the QKV projection of stage s+1 and the
   out-projection of stage s-1 are interleaved into stage s's attention at
   k-tile granularity, so the PE never sits idle waiting for the softmax
   exp chain (ACT) and the ACT is busy from the first stage on.
 - ScalarE (ACT) runs ONLY the exps; all PSUM evacuation is on DVE.
 - Causal masking via gpsimd affine_select (iota vs partition id) instead of
   DVE band-mask multiplies — frees DVE, no mask constant needed.
 - Diagonal score/AV matmuls are narrowed to the causal width, floored at 256
   columns (fp32r matmuls stream at 1/4 rate below 256 moving columns).

All matmuls run as float32r (fp32 data, reduced-precision PE mode).

Attention layout (transpose-free):
  qT, kT   [64d x L] per head (2 heads stacked per 128 partitions)
  S^T tile [128k x w_q] = kT_tile.T @ qT_block  (PE, K=64, 2 heads row-tiled)
  expS     = exp(S^T)  (ACT, PSUM->SBUF), diag tiles causal-zeroed on gpsimd
  O^T,sums [65 x 512q] += [V_tile | ones].T-form @ expS  (PE, K=128)
  O^T_norm = O^T * broadcast(1/sums)  -> directly the lhsT of out-proj
  Y tile   [128l x 512e] = sum_pairs O^T_pair.T @ Wo_pair
"""

import os
from collections import deque
from contextlib import ExitStack

import numpy as np

os.environ.setdefault("JAX_PLATFORMS", "")

import concourse.bass as bass
import concourse.mybir as mybir
import concourse.tile as tile
from concourse import bacc, bass_utils

F32 = mybir.dt.float32
F32R = mybir.dt.float32r
AF = mybir.ActivationFunctionType

B, L, D, H = 4, 2048, 1024, 16
DK = D // H            # 64
G = 2                  # head groups (tensor parallel)
HPG = H // G           # 8 heads per group
GW = HPG * DK          # 512 columns per group
P = 128
CO = D // P            # 8 contraction tiles for projections
LT = L // P            # 16 l-tiles / k-tiles
QW = 512               # q-block width
QB = L // QW           # 4 q-blocks / stages
NPAIR = HPG // 2       # 4 head-pairs per group (2 heads per 128 partitions)
XC = 256               # xt load chunk width (2 chunks per stage)

_NC_CACHE: dict = {}


def build_nc(with_qk_bias: bool, repeat: int = 1):
    nc = bacc.Bacc("TRN2", target_bir_lowering=False, debug=False, num_devices=8)

    xt = nc.dram_tensor("xt", [D, L], F32, kind="ExternalInput").ap()
    wq = nc.dram_tensor("wq", [D, GW], F32, kind="ExternalInput").ap()
    wk = nc.dram_tensor("wk", [D, GW], F32, kind="ExternalInput").ap()
    wv = nc.dram_tensor("wv", [D, GW], F32, kind="ExternalInput").ap()
    wo = nc.dram_tensor("wo", [GW, D], F32, kind="ExternalInput").ap()
    if with_qk_bias:
        bq = nc.dram_tensor("bq", [P, NPAIR], F32, kind="ExternalInput").ap()
        bk = nc.dram_tensor("bk", [P, NPAIR], F32, kind="ExternalInput").ap()
    y = nc.dram_tensor("y", [L, D], F32, kind="ExternalOutput").ap()

    xt_r = xt.rearrange("(co p) l -> co p l", p=P)
    wq_r = wq.rearrange("(co p) c -> co p c", p=P)
    wk_r = wk.rearrange("(co p) c -> co p c", p=P)
    wv_r = wv.rearrange("(co p) c -> co p c", p=P)
    wo_r = wo.rearrange("(pr p) e -> pr p e", p=P)
    y_r = y.rearrange("(lt p) e -> lt p e", p=P)

    with tile.TileContext(nc) as tc, ExitStack() as ctx:
        wp = ctx.enter_context(tc.tile_pool(name="w", bufs=1))
        xtp = ctx.enter_context(tc.tile_pool(name="xt", bufs=3))
        qkp = ctx.enter_context(tc.tile_pool(name="qk", bufs=1))
        vp = ctx.enter_context(tc.tile_pool(name="v", bufs=1))
        esp = ctx.enter_context(tc.tile_pool(name="es", bufs=2))
        otp = ctx.enter_context(tc.tile_pool(name="ot", bufs=2))
        ybp = ctx.enter_context(tc.tile_pool(name="yb", bufs=2))
        nrm = ctx.enter_context(tc.tile_pool(name="nrm", bufs=1))
        psp = ctx.enter_context(tc.tile_pool(name="ps", bufs=2, space="PSUM"))
        cstp = ctx.enter_context(tc.tile_pool(name="cst", bufs=1)) \
            if with_qk_bias else None

        for _rep in range(repeat):
            _kernel_body(nc, tc, with_qk_bias, locals())

    nc.compile()
    return nc


def _kernel_body(nc, tc, with_qk_bias, env):
    wp, xtp, qkp, vp, esp, otp, ybp, nrm, psp, cstp = (
        env["wp"], env["xtp"], env["qkp"], env["vp"], env["esp"],
        env["otp"], env["ybp"], env["nrm"], env["psp"], env["cstp"])
    xt_r, wq_r, wk_r, wv_r, wo_r, y_r = (
        env["xt_r"], env["wq_r"], env["wk_r"], env["wv_r"],
        env["wo_r"], env["y_r"])
    mm = nc.tensor.matmul

    # ---- persistent tiles (tag rings across repeats) ----
    wq_sb = wp.tile([P, CO, GW], F32R, tag="wq")
    wk_sb = wp.tile([P, CO, GW], F32R, tag="wk")
    wv_sb = wp.tile([P, CO, GW], F32R, tag="wv")
    wo_sb = wp.tile([P, NPAIR, D], F32R, tag="wo")
    kT = qkp.tile([P, NPAIR, L], F32R, tag="kT")
    vext = vp.tile([P, LT, HPG, DK + 1], F32R, tag="vext")

    chunks: dict = {}

    def load_xt(s):
        for h in range(2):
            t = xtp.tile([P, CO, XC], F32R, tag="xt", name=f"xt{s}{h}")
            base = s * QW + h * XC
            for co in range(CO):
                nc.sync.dma_start(
                    t[:, co], xt_r[co, :, base:base + XC].bitcast(F32R))
            chunks[(s, h)] = t

    # co-interleaved so the first Q matmul can start after the first pair of
    # transfers lands (subtile deps), not after the whole prologue DMA train
    t0 = xtp.tile([P, CO, XC], F32R, tag="xt", name="xt00")
    t1 = xtp.tile([P, CO, XC], F32R, tag="xt", name="xt01")
    chunks[(0, 0)], chunks[(0, 1)] = t0, t1
    for co in range(CO):
        nc.sync.dma_start(t0[:, co], xt_r[co, :, 0:XC].bitcast(F32R))
        nc.sync.dma_start(wq_sb[:, co], wq_r[co].bitcast(F32R))
    for co in range(CO):
        nc.sync.dma_start(t1[:, co], xt_r[co, :, XC:2 * XC].bitcast(F32R))
        nc.sync.dma_start(wk_sb[:, co], wk_r[co].bitcast(F32R))
    for co in range(CO):
        nc.sync.dma_start(wv_sb[:, co], wv_r[co].bitcast(F32R))
    for pair in range(NPAIR):
        nc.sync.dma_start(wo_sb[:, pair], wo_r[pair].bitcast(F32R))
    if with_qk_bias:
        bq_sb = cstp.tile([P, NPAIR], F32, tag="bq")
        bk_sb = cstp.tile([P, NPAIR], F32, tag="bk")
        nc.sync.dma_start(bq_sb[:], env["bq"])
        nc.sync.dma_start(bk_sb[:], env["bk"])

    # softmax-denominator ones column of [V | 1]: memset the whole tile
    # (contiguous — a strided column memset fails the ISA check); the V
    # evacuations then overwrite the [0:DK] value columns.
    nc.vector.memset(vext[:].bitcast(F32), 1.0)

    qT_of: dict = {}
    oT_of: dict = {}

    # ---------------- work groups (emitted via the fill queue) ----------------
    def qk_group(s, which, pair):
        def g():
            w_sb, dst_bias = (wq_sb, "bq") if which == "q" else (wk_sb, "bk")
            pt = psp.tile([P, QW], F32, tag="pq", name="pt")
            for h in range(2):
                for co in range(CO):
                    mm(pt[:, h * XC:(h + 1) * XC],
                       w_sb[:, co, pair * P:(pair + 1) * P],
                       chunks[(s, h)][:, co],
                       start=(h == 0 and co == 0),
                       stop=(h == 1 and co == CO - 1))
            if which == "q":
                if s not in qT_of:
                    qT_of[s] = qkp.tile([P, NPAIR, QW], F32R, tag="qT",
                                        bufs=2, name=f"qT{s}")
                dst = qT_of[s][:, pair, :]
            else:
                dst = kT[:, pair, s * QW:(s + 1) * QW]
            if with_qk_bias:
                bt = env["bq_sb"] if dst_bias == "bq" else env["bk_sb"]
                nc.vector.tensor_scalar_add(dst, pt[:], bt[:, pair:pair + 1])
            else:
                nc.vector.tensor_copy(dst, pt[:])
        return g

    def v_group(s, i):
        def g():
            lt = 4 * s + i
            ch = chunks[(s, i // 2)]
            pv = psp.tile([P, GW], F32, tag="pq", name="pv")
            for co in range(CO):
                mm(pv[:], ch[:, co, (i % 2) * P:(i % 2 + 1) * P], wv_sb[:, co],
                   start=co == 0, stop=co == CO - 1)
            nc.vector.tensor_copy(
                vext[:, lt, :, 0:DK],
                pv[:].rearrange("p (h d) -> p h d", h=HPG))
        return g

    def out_group(s, i):
        def g():
            lt = 4 * s + i
            oTs = oT_of[s]
            for eh in range(2):
                py = psp.tile([P, QW], F32, tag="pq", name="py")
                for pair in range(NPAIR):
                    mm(py[:], oTs[:, pair, i * P:(i + 1) * P],
                       wo_sb[:, pair, eh * QW:(eh + 1) * QW],
                       start=pair == 0, stop=pair == NPAIR - 1)
                yb = ybp.tile([P, QW], F32, tag="yb", name="yb")
                nc.vector.tensor_copy(yb[:], py[:])
                nc.sync.dma_start(y_r[lt][:, eh * QW:(eh + 1) * QW], yb[:])
        return g

    def qkv_groups(s):
        gs = [qk_group(s, "q", pair) for pair in range(NPAIR)]
        gs += [qk_group(s, "k", pair) for pair in range(NPAIR)]
        gs += [v_group(s, i) for i in range(4)]
        return gs

    fill_q: deque = deque()

    def fill_tick(n=1):
        for _ in range(n):
            if fill_q:
                fill_q.popleft()()

    # ---------------- prologue: stage-0 projection ----------------
    for g in qkv_groups(0):
        g()

    # ---------------- fused attention pipeline ----------------
    for s in range(QB):
        if s + 1 < QB:
            load_xt(s + 1)
            fill_q.extend(qkv_groups(s + 1))
        if s - 1 >= 0:
            fill_q.extend(out_group(s - 1, i) for i in range(4))
        qTs = qT_of[s]
        oTs = otp.tile([P, NPAIR, QW], F32R, tag="oT", name=f"oT{s}")
        oT_of[s] = oTs
        nj = 4 * s + 4
        for p in range(NPAIR):
            po = [psp.tile([DK + 1, QW], F32, tag=f"po{u}", bufs=1,
                           name=f"po{u}") for u in range(2)]
            es_of = {}

            def emit_scores(j, p=p, qTs=qTs, es_of=es_of):
                # scores+exp+mask for k-tile j, emitted one j AHEAD of the AV
                # mms: otherwise S(j+1) sits behind AV(j)+fill in the in-order
                # PE queue, the ACT starves, and the exp chain paces the body
                o = j - 4 * s
                if o < 0:
                    qs, w = 0, QW
                elif o < 2:
                    qs, w = o * P, QW - o * P
                else:
                    qs, w = 2 * P, 2 * P
                ps = psp.tile([P, 2, QW], F32, tag="s", name="ps")
                es = esp.tile([P, 2, QW], F32R, tag="es", name="es")
                for u in range(2):
                    mm(ps[:, u, qs:qs + w],
                       kT[u * DK:(u + 1) * DK, p, j * P:(j + 1) * P],
                       qTs[u * DK:(u + 1) * DK, p, qs:qs + w],
                       start=True, stop=True, tile_position=(u * DK, 0))
                nc.scalar.activation(es[:, :, qs:qs + w],
                                     ps[:, :, qs:qs + w], AF.Exp)
                if o >= 0:
                    # causal zeroing of the diagonal band: keep iff q >= k,
                    # i.e. col_iota - k >= 0 (channel_multiplier = -1)
                    if o < 3:
                        sl = es[:, :, o * P:(o + 1) * P]
                        base, bw = 0, P
                    else:
                        sl = es[:, :, 2 * P:4 * P]
                        base, bw = -P, 2 * P
                    nc.gpsimd.affine_select(
                        sl, sl, pattern=[[0, 2], [1, bw]],
                        compare_op=mybir.AluOpType.is_ge, fill=0.0,
                        base=base, channel_multiplier=-1)
                es_of[j] = (es, qs, w)

            emit_scores(0)
            for j in range(nj):
                if j + 1 < nj:
                    emit_scores(j + 1)
                fill_tick()
                es, qs, w = es_of.pop(j)
                for u in range(2):
                    mm(po[u][0:DK + 1, qs:qs + w],
                       vext[:, j, 2 * p + u, :],
                       es[:, u, qs:qs + w],
                       start=j == 0, stop=j == nj - 1)
            # stage O^T|sums to SBUF immediately so the po PSUM banks free up
            # for the next pair's AV after 2 copies, not the whole norm chain
            posb = [nrm.tile([DK + 1, QW], F32, tag=f"posb{u}",
                             name=f"posb{u}") for u in range(2)]
            for u in range(2):
                nc.vector.tensor_copy(posb[u][:], po[u][0:DK + 1, :])
            fill_tick()
            # normalization: oT = O^T * broadcast(1/sums)
            for u in range(2):
                sm = nrm.tile([1, QW], F32, tag="sm", name="sm")
                nc.vector.tensor_copy(sm[:], posb[u][DK:DK + 1, :])
                rc = nrm.tile([1, QW], F32, tag="rc", name="rc")
                nc.vector.reciprocal_approx_fast(rc[:], sm[:])
                rcb = nrm.tile([DK, QW], F32, tag="rcb", name="rcb")
                nc.gpsimd.partition_broadcast(rcb[:], rc[:])
                nc.vector.tensor_mul(oTs[u * DK:(u + 1) * DK, p, :],
                                     posb[u][0:DK, :], rcb[:])
            fill_tick()
        while fill_q:
            fill_tick()
    for i in range(4):
        out_group(QB - 1, i)()


def _prep_inputs(x, W_qkv, b_qkv, W_out):
    """Per-core input maps. Core c -> batch c//2, head-group c%2."""
    x = np.ascontiguousarray(np.asarray(x, dtype=np.float32))
    W_qkv = np.asarray(W_qkv, dtype=np.float32)
    b_qkv = np.asarray(b_qkv, dtype=np.float32)
    W_out = np.asarray(W_out, dtype=np.float32)

    scale = 1.0 / np.sqrt(DK)
    with_qk_bias = bool(np.any(b_qkv[:2 * D]))
    xts = [np.ascontiguousarray(x[b].T) for b in range(B)]
    in_maps = []
    for c in range(8):
        b, g = c // 2, c % 2
        sl = slice(g * GW, (g + 1) * GW)
        m = {
            "xt": xts[b],
            "wq": np.ascontiguousarray(W_qkv[:, g * GW:(g + 1) * GW]) * scale,
            "wk": np.ascontiguousarray(W_qkv[:, D + g * GW:D + (g + 1) * GW]),
            "wv": np.ascontiguousarray(W_qkv[:, 2 * D + g * GW:2 * D + (g + 1) * GW]),
            "wo": np.ascontiguousarray(W_out[sl, :]),
        }
        if with_qk_bias:
            m["bq"] = np.ascontiguousarray(
                b_qkv[g * GW:(g + 1) * GW].reshape(NPAIR, P).T) * scale
            m["bk"] = np.ascontiguousarray(
                b_qkv[D + g * GW:D + (g + 1) * GW].reshape(NPAIR, P).T)
        in_maps.append(m)
    return in_maps, with_qk_bias


def kernel(x, W_qkv, b_qkv, W_out, b_out):
    in_maps, with_qk_bias = _prep_inputs(x, W_qkv, b_qkv, W_out)

    key = ("nc", with_qk_bias)
    if key not in _NC_CACHE:
        _NC_CACHE[key] = build_nc(with_qk_bias)
    nc = _NC_CACHE[key]

    res = bass_utils.run_bass_kernel_spmd(nc, in_maps, core_ids=list(range(8)))
    parts = [r["y"] for r in res.results]

    b_qkv = np.asarray(b_qkv, dtype=np.float32)
    W_out_np = np.asarray(W_out, dtype=np.float32)
    # v-bias passes through attention (rows of attn sum to 1) and out-proj is
    # linear: contribution = b_v @ W_out; b_out adds directly.
    corr = (b_qkv[2 * D:3 * D] @ W_out_np
            + np.asarray(b_out, dtype=np.float32)).astype(np.float32)

    out = np.empty((B, L, D), dtype=np.float32)
    for b in range(B):
        out[b] = parts[2 * b] + parts[2 * b + 1] + corr
    return out
